# revision 1
# baseline (speedup 1.0000x reference)
"""Causal self-attention (B=2, S=2048, D=2048, H=16) on 8 trn2 NeuronCores.

Sharding: core c -> batch b = c//4, head-group hg = c%4 (4 heads of 128 dims).
Each core computes its heads' attention plus the partial output projection
(row-parallel split of W_proj); the host sums the 4 partials per batch.

One fused emission stream, organized in 512-query stripes.  Emission blocks:
  b0: qk-proj(0) (k-outer, DMA-paced) + qk-proj(1) + v-proj(0)
  b1: attn(0) interleaved with qk(2)+v(1)
  b2: attn(1) interleaved with qk(3)+v(2)+outproj(0)
  b3: attn(2) interleaved with v(3)
  b4: attn(3) interleaved with outproj(1)+outproj(2)
  tail: outproj(3)
Filler matmuls between attention slots hide the scalar-engine exp latency
(~660ns/tile vs ~430ns of PE work per slot).

Dtypes: Q/K projections run fp8-e4m3 DoubleRow (2 contraction rows/cycle);
everything else bf16 (fp32 PSUM accumulation throughout).  Wq/Wk are
pre-scaled by 16 into fp8 normal range; the 16*16 score scale is divided
back out inside the exp's scale constant.
"""

import sys

sys.path.insert(0, "/opt/trn_rl_repo")

from contextlib import ExitStack

import numpy as np
import ml_dtypes

import concourse.bass as bass
import concourse.mybir as mybir
import concourse.tile as tile
from concourse import bacc
from concourse.bass_utils import run_bass_kernel_spmd

B, S, D, H = 2, 2048, 2048, 16
HD = D // H  # 128
NH = 4  # heads per core
HG = H // NH  # head groups = 4
P = 128
KT = D // P  # 16 k-subtiles over model dim
NQ = 4  # 512-query stripes
QW = S // NQ  # 512
ST = S // P  # 16 s-tiles of 128
SCALE = float(1.0 / np.sqrt(D).astype(np.float32))
WS = 16.0  # Wq/Wk fp8 pre-scale
ESCALE = SCALE / (WS * WS)
MASK_NEG = -30000.0 * WS * WS  # exp(ESCALE * (s + MASK_NEG)) == 0

F32 = mybir.dt.float32
BF16 = mybir.dt.bfloat16
FP8 = mybir.dt.float8e4

USE_FP8 = True


def build_bass(use_fp8=USE_FP8):
    nc = bacc.Bacc("TRN2")

    # x arrives stripe-blocked ([NQ*P, KT*QW]: stripe-major, partition-major,
    # k-major) so every stripe load is one contiguous 8/16KB run per
    # partition — strided [D,S] slices only manage ~150GB/s.
    xb = nc.declare_dram_parameter("xb", [NQ * P, KT * QW], BF16, isOutput=False)
    # wq/wk arrive head-major ([NH*D, HD]) so per-head loads are contiguous
    # and the first projection tiles start after ~0.5MB of weights.
    if use_fp8:
        x8 = nc.declare_dram_parameter("x8", [NQ * P, KT * QW], FP8, isOutput=False)
        wq = nc.declare_dram_parameter("wq", [NH * D, HD], FP8, isOutput=False)
        wk = nc.declare_dram_parameter("wk", [NH * D, HD], FP8, isOutput=False)
    else:
        x8 = xb
        wq = nc.declare_dram_parameter("wq", [NH * D, HD], BF16, isOutput=False)
        wk = nc.declare_dram_parameter("wk", [NH * D, HD], BF16, isOutput=False)
    wv = nc.declare_dram_parameter("wv", [D, NH * HD], BF16, isOutput=False)
    wp = nc.declare_dram_parameter("wp", [NH * HD, D], BF16, isOutput=False)
    maskp = nc.declare_dram_parameter("maskp", [P, P], F32, isOutput=False)
    y = nc.declare_dram_parameter("y", [S, D], BF16, isOutput=True)

    qk_dt = FP8 if use_fp8 else BF16
    DR = mybir.MatmulPerfMode.DoubleRow if use_fp8 else None
    kstep = 2 if use_fp8 else 1  # contraction subtiles per QK-proj matmul

    def xblock(xB, n, ks=None):
        src = xB[n * P : (n + 1) * P, :].rearrange("p (k w) -> p k w", k=KT)
        return src if ks is None else src[:, ks, :]
    wqr = wq[:, :].rearrange("(h k p) m -> p h k m", h=NH, p=P)
    wkr = wk[:, :].rearrange("(h k p) m -> p h k m", h=NH, p=P)
    wvr = wv[:, :].rearrange("(k p) m -> p k m", p=P)
    wpr = wp[:, :].rearrange("(h p) n -> p h n", p=P)

    with tile.TileContext(nc) as tc, ExitStack() as top:
        const = top.enter_context(tc.tile_pool(name="const", bufs=1))
        persist = top.enter_context(tc.tile_pool(name="persist", bufs=1))
        xpool = top.enter_context(tc.tile_pool(name="xpool", bufs=2))
        qpool = top.enter_context(tc.tile_pool(name="qpool", bufs=3))
        epool = top.enter_context(tc.tile_pool(name="epool", bufs=6))
        espool = top.enter_context(tc.tile_pool(name="espool", bufs=2))
        rpool = top.enter_context(tc.tile_pool(name="rpool", bufs=2))
        ypool = top.enter_context(tc.tile_pool(name="ypool", bufs=2))
        ppool = top.enter_context(tc.tile_pool(name="ppsum", bufs=3, space="PSUM"))
        spool = top.enter_context(tc.tile_pool(name="spsum", bufs=2, space="PSUM"))
        upool = top.enter_context(tc.tile_pool(name="upsum", bufs=2, space="PSUM"))
        dpool = top.enter_context(tc.tile_pool(name="dpsum", bufs=1, space="PSUM"))

        mask_sb = const.tile([P, P], F32)
        ones_f32 = const.tile([P, 1], F32)
        nc.vector.memset(ones_f32, 1.0)
        ones_col = const.tile([P, 1], BF16)
        nc.vector.tensor_copy(ones_col, ones_f32)

        # weights resident in SBUF for the whole kernel
        wq_sb = persist.tile([P, NH, KT, HD], qk_dt, tag="wq")
        wk_sb = persist.tile([P, NH, KT, HD], qk_dt, tag="wk")
        wv_sb = persist.tile([P, KT, NH * HD], BF16, tag="wv")
        wp_sb = persist.tile([P, NH, D], BF16, tag="wp")
        # K^T [HD, S] per head; V [keys, feats] stripes; normalized attn out^T
        kT = persist.tile([P, NH, S], BF16, tag="kT")
        vA = persist.tile([P, ST, NH * HD], BF16, tag="vA")
        aA = persist.tile([P, NH, S], BF16, tag="aA")

        x8_blk = [None] * NQ
        xb_blk = [None] * NQ
        qT = [[None] * NH for _ in range(NQ)]  # [n][h] -> [HD, QW] bf16

        def alloc_x8(n):
            x8_blk[n] = xpool.tile([P, KT, QW], qk_dt, tag="x8", name=f"x8_{n}")
            return x8_blk[n]

        def alloc_xb(n):
            xb_blk[n] = xpool.tile([P, KT, QW], BF16, tag="xb", name=f"xb_{n}")
            return xb_blk[n]

        # block-0 critical loads, interleaved in consumption order so the
        # first matmuls start ~2us in and stream behind the DMA
        alloc_x8(0)
        nc.sync.dma_start(wq_sb[:, 0], wqr[:, 0])
        nc.sync.dma_start(wk_sb[:, 0], wkr[:, 0])
        for c in range(0, KT, 4):
            ks = slice(c, c + 4)
            nc.sync.dma_start(x8_blk[0][:, ks, :], xblock(x8, 0, ks))
        for h in range(1, NH):
            nc.sync.dma_start(wq_sb[:, h], wqr[:, h])
            nc.sync.dma_start(wk_sb[:, h], wkr[:, h])
        nc.sync.dma_start(mask_sb, maskp[:, :])  # first needed by attn(0) in b1

        # ---------- emission units ----------

        def dma_unit(fn):
            return [fn]

        def qk_units(n, k_outer=False):
            """Q/K projection for stripe n: 8 psum tiles of 8 DR matmuls.

            k_outer=True emits tile pairs k-first so the first stripe paces
            the chunked DMA stream instead of stalling on the full tensor.
            """
            units = []
            tiles = []
            for h in range(NH):
                for w_sb, is_q in ((wq_sb, True), (wk_sb, False)):
                    tiles.append((h, w_sb, is_q))

            def bounce(st, h, is_q):
                if is_q:
                    qt = qpool.tile([P, QW], BF16, tag=f"q{h}", name=f"qT_{n}_{h}")
                    qT[n][h] = qt
                    nc.vector.tensor_copy(qt, st["ps"])
                else:
                    nc.vector.tensor_copy(kT[:, h, n * QW : (n + 1) * QW], st["ps"])

            if k_outer:
                for t0 in range(0, len(tiles), 2):
                    pair = tiles[t0 : t0 + 2]
                    st = [{}, {}]

                    def open_pair(pair=pair, st=st):
                        for i, (h, w_sb, is_q) in enumerate(pair):
                            st[i]["ps"] = ppool.tile([P, QW], F32, tag="p", name="psqk")
                        for k in range(0, KT // 2, kstep):
                            for i, (h, w_sb, is_q) in enumerate(pair):
                                nc.tensor.matmul(
                                    st[i]["ps"],
                                    lhsT=w_sb[:, h, k : k + kstep, :],
                                    rhs=x8_blk[n][:, k : k + kstep, :],
                                    start=(k == 0),
                                    stop=False,
                                    perf_mode=DR,
                                )

                    def close_pair(pair=pair, st=st):
                        for k in range(KT // 2, KT, kstep):
                            for i, (h, w_sb, is_q) in enumerate(pair):
                                nc.tensor.matmul(
                                    st[i]["ps"],
                                    lhsT=w_sb[:, h, k : k + kstep, :],
                                    rhs=x8_blk[n][:, k : k + kstep, :],
                                    start=False,
                                    stop=(k + kstep == KT),
                                    perf_mode=DR,
                                )
                        for i, (h, w_sb, is_q) in enumerate(pair):
                            bounce(st[i], h, is_q)

                    units.append(open_pair)
                    units.append(close_pair)
                return units

            for h, w_sb, is_q in tiles:
                st = {}

                def open_tile(st=st, w_sb=w_sb, h=h):
                    st["ps"] = ppool.tile([P, QW], F32, tag="p", name="psqk")
                    for k in range(0, KT // 2, kstep):
                        nc.tensor.matmul(
                            st["ps"],
                            lhsT=w_sb[:, h, k : k + kstep, :],
                            rhs=x8_blk[n][:, k : k + kstep, :],
                            start=(k == 0),
                            stop=False,
                            perf_mode=DR,
                        )

                def close_tile(st=st, w_sb=w_sb, h=h, is_q=is_q):
                    for k in range(KT // 2, KT, kstep):
                        nc.tensor.matmul(
                            st["ps"],
                            lhsT=w_sb[:, h, k : k + kstep, :],
                            rhs=x8_blk[n][:, k : k + kstep, :],
                            start=False,
                            stop=(k + kstep == KT),
                            perf_mode=DR,
                        )
                    bounce(st, h, is_q)

                units.append(open_tile)
                units.append(close_tile)
            return units

        def v_units(n, k_outer=False):
            """V projection for stripe n: 4 psum tiles of 16 matmuls."""
            units = []
            if k_outer:
                for m0 in (0, 2):
                    st = {}

                    def emitk(kk, st=st, m0=m0):
                        def emit():
                            if kk == 0:
                                st["ps"] = [
                                    ppool.tile([P, QW], F32, tag="p", name="psv")
                                    for _ in range(2)
                                ]
                            for k in range(kk, kk + 4):
                                for i in range(2):
                                    nc.tensor.matmul(
                                        st["ps"][i],
                                        lhsT=xb_blk[n][
                                            :, k, (m0 + i) * P : (m0 + i + 1) * P
                                        ],
                                        rhs=wv_sb[:, k, :],
                                        start=(k == 0),
                                        stop=(k == KT - 1),
                                    )
                            if kk + 4 == KT:
                                for i in range(2):
                                    nc.vector.tensor_copy(
                                        vA[:, n * 4 + m0 + i, :], st["ps"][i]
                                    )

                        return emit

                    units.extend(emitk(kk) for kk in range(0, KT, 4))
                return units

            for mi in range(4):
                st = {}

                def open_v(st=st, mi=mi):
                    st["ps"] = ppool.tile([P, QW], F32, tag="p", name="psv")
                    for k in range(KT // 2):
                        nc.tensor.matmul(
                            st["ps"],
                            lhsT=xb_blk[n][:, k, mi * P : (mi + 1) * P],
                            rhs=wv_sb[:, k, :],
                            start=(k == 0),
                            stop=False,
                        )

                def close_v(st=st, mi=mi):
                    for k in range(KT // 2, KT):
                        nc.tensor.matmul(
                            st["ps"],
                            lhsT=xb_blk[n][:, k, mi * P : (mi + 1) * P],
                            rhs=wv_sb[:, k, :],
                            start=False,
                            stop=(k == KT - 1),
                        )
                    nc.vector.tensor_copy(vA[:, n * 4 + mi, :], st["ps"])

                units.append(open_v)
                units.append(close_v)
            return units

        def attn_units(qg):
            """Attention chains for query stripe qg, heads sequential.

            Unit j emits score(j) then PV(j-1): the exp(j-1) on the scalar
            engine hides behind score(j) plus whatever filler the merge puts
            between units.
            """
            units = []
            kmax = 4 * qg + 4
            for h in range(NH):
                st = {"sps": [None] * kmax, "e": [None] * kmax}

                def slot(j, st=st, h=h, qg=qg, kmax=kmax):
                    def emit():
                        if j < kmax:
                            r = j - 4 * qg
                            c0 = max(0, r) * P
                            sps = spool.tile([P, QW], F32, tag="s", name="sps")
                            st["sps"][j] = sps
                            nc.tensor.matmul(
                                sps[:, c0:],
                                lhsT=kT[:, h, j * P : (j + 1) * P],
                                rhs=qT[qg][h][:, c0:],
                                start=True,
                                stop=True,
                            )
                            if r >= 0:
                                nc.vector.tensor_tensor(
                                    sps[:, c0 : c0 + P],
                                    sps[:, c0 : c0 + P],
                                    mask_sb,
                                    op=mybir.AluOpType.add,
                                )
                            e = epool.tile([P, QW], BF16, tag="e", name="e")
                            st["e"][j] = e
                            nc.scalar.activation(
                                e[:, c0:],
                                sps[:, c0:],
                                mybir.ActivationFunctionType.Exp,
                                scale=ESCALE,
                            )
                            if j == 0:
                                st["esum"] = espool.tile(
                                    [P, QW], BF16, tag="es", name="esum"
                                )
                                nc.vector.tensor_copy(st["esum"], e)
                            else:
                                nc.vector.tensor_tensor(
                                    st["esum"][:, c0:],
                                    st["esum"][:, c0:],
                                    e[:, c0:],
                                    op=mybir.AluOpType.add,
                                )
                        if j > 0:
                            jj = j - 1
                            r = jj - 4 * qg
                            c0 = max(0, r) * P
                            if jj == 0:
                                st["ups"] = upool.tile([P, QW], F32, tag="u", name="ups")
                            nc.tensor.matmul(
                                st["ups"][:, c0:],
                                lhsT=vA[:, jj, h * HD : (h + 1) * HD],
                                rhs=st["e"][jj][:, c0:],
                                start=(jj == 0),
                                stop=(jj == kmax - 1),
                            )
                            st["e"][jj] = None
                            st["sps"][jj] = None
                        if j == kmax:
                            dps = dpool.tile([1, QW], F32, tag="d", name="dps")
                            nc.tensor.matmul(
                                dps, lhsT=ones_col, rhs=st["esum"],
                                start=True, stop=True,
                            )
                            rcp = rpool.tile([1, QW], F32, tag="rcp", name="rcp")
                            nc.vector.reciprocal_approx_fast(rcp, dps)
                            rb = rpool.tile([P, QW], F32, tag="rb", name="rb")
                            nc.gpsimd.partition_broadcast(rb, rcp)
                            nc.vector.tensor_tensor(
                                aA[:, h, qg * QW : (qg + 1) * QW],
                                st["ups"],
                                rb,
                                op=mybir.AluOpType.mult,
                            )

                    return emit

                units.extend(slot(j) for j in range(kmax + 1))
            return units

        def p3_units(qg, cast_engines=(None,), per_cc_store=False):
            """Output projection rows m in stripe qg: y[m*128:, :] = sum_h a_h^T wp_h.

            cast_engines picks which engine(s) run the PSUM->bf16 bounce
            (round-robin) so neither the vector nor the scalar queue
            saturates during the last attention stripe.  gpsimd can't read
            PSUM; it only kicks the store DMA.
            """
            units = []
            ci = 0
            for mi in range(4):
                m = qg * 4 + mi
                st = {}

                def open_y(st=st):
                    st["yb"] = ypool.tile([P, NQ * QW], BF16, tag="yb", name="yb")

                def unit(cc, st=st, m=m, ci=ci):
                    def emit():
                        yp = ppool.tile([P, QW], F32, tag="p", name="psy")
                        for h in range(NH):
                            nc.tensor.matmul(
                                yp,
                                lhsT=aA[:, h, m * P : (m + 1) * P],
                                rhs=wp_sb[:, h, cc * QW : (cc + 1) * QW],
                                start=(h == 0),
                                stop=(h == NH - 1),
                            )
                        eng = cast_engines[ci % len(cast_engines)]
                        dst = st["yb"][:, cc * QW : (cc + 1) * QW]
                        if eng == "scalar":
                            nc.scalar.copy(dst, yp)
                        else:
                            nc.vector.tensor_copy(dst, yp)
                        if per_cc_store:
                            nc.gpsimd.dma_start(
                                y[m * P : (m + 1) * P, cc * QW : (cc + 1) * QW], dst
                            )
                        elif cc == NQ - 1:
                            nc.gpsimd.dma_start(
                                y[m * P : (m + 1) * P, :], st["yb"]
                            )

                    return emit

                units.append(open_y)
                for cc in range(NQ):
                    units.append(unit(cc, ci=ci))
                    ci += 1
            return units

        def merge(attn, other, reserve=0):
            """Interleave: spread `other` between attention units, holding
            back `reserve` units to emit after the last attention unit (keeps
            the PE busy through the final softmax-normalize chain)."""
            if not attn:
                for u in other:
                    u()
                return
            spread = other[: len(other) - reserve] if reserve else other
            held = other[len(other) - reserve :] if reserve else []
            ratio = len(spread) / len(attn)
            acc = 0.0
            oi = 0
            for u in attn:
                u()
                acc += ratio
                while acc >= 1.0 and oi < len(spread):
                    spread[oi]()
                    oi += 1
                    acc -= 1.0
            while oi < len(spread):
                spread[oi]()
                oi += 1
            for u in held:
                u()

        # ---------- pipeline ----------
        # b0: qk(0) k-outer (paced by the chunked DMA above), qk(1), v(0)
        for u in qk_units(0, k_outer=True):
            u()
        nc.sync.dma_start(alloc_x8(1), xblock(x8, 1))
        alloc_xb(0)
        for c in range(0, KT, 4):
            ks = slice(c, c + 4)
            nc.sync.dma_start(wv_sb[:, ks, :], wvr[:, ks, :])
            nc.sync.dma_start(xb_blk[0][:, ks, :], xblock(xb, 0, ks))
        nc.sync.dma_start(wp_sb, wpr)
        for u in qk_units(1):
            u()
        for u in v_units(0, k_outer=True):
            u()
        nc.sync.dma_start(alloc_x8(2), xblock(x8, 2))

        # b1
        nc.sync.dma_start(alloc_xb(1), xblock(xb, 1))
        nc.sync.dma_start(alloc_x8(3), xblock(x8, 3))
        merge(attn_units(0), qk_units(2) + v_units(1), reserve=2)
        # b2
        nc.sync.dma_start(alloc_xb(2), xblock(xb, 2))
        merge(
            attn_units(1),
            qk_units(3) + v_units(2) + p3_units(0, cast_engines=("scalar",)),
            reserve=4,
        )
        # b3
        nc.sync.dma_start(alloc_xb(3), xblock(xb, 3))
        merge(attn_units(2), v_units(3), reserve=2)
        # b4
        # p3(1) last: the reserved units' casts run on the scalar queue, so
        # the final chain's reciprocal/normalize aren't stuck behind casts
        # in the vector FIFO while the PE waits on them.
        merge(
            attn_units(3),
            p3_units(2, cast_engines=("vector",))
            + p3_units(1, cast_engines=("scalar",)),
            reserve=8,
        )
        # tail
        for u in p3_units(3, cast_engines=("vector", "scalar"), per_cc_store=True):
            u()

    nc.finalize()
    return nc


def _build_mask():
    # tri block mask for diagonal tiles: allowed iff query-col >= key-row.
    # Added to the raw (x16-scaled) scores before exp(ESCALE * .).
    k = np.arange(P)[:, None]
    c = np.arange(P)[None, :]
    return np.where(c >= k, 0.0, MASK_NEG).astype(np.float32)


_NC_CACHE = {}


def _get_nc():
    if "nc" not in _NC_CACHE:
        _NC_CACHE["nc"] = build_bass()
    return _NC_CACHE["nc"]


def make_in_maps(x, W_qkv, W_proj):
    x = np.asarray(x, dtype=np.float32)
    W_qkv = np.asarray(W_qkv, dtype=np.float32)
    W_proj = np.asarray(W_proj, dtype=np.float32)
    Wq, Wk, Wv = W_qkv[0:D], W_qkv[D : 2 * D], W_qkv[2 * D : 3 * D]
    mask = _build_mask()
    bf16 = ml_dtypes.bfloat16
    fp8 = ml_dtypes.float8_e4m3
    def stripe_block(t):
        # [D, S] -> [NQ*P, KT*QW]: stripe n, partition p holds the k-major
        # contiguous slab t[k*128+p, n*512:(n+1)*512] for all k
        return np.ascontiguousarray(
            t.reshape(KT, P, NQ, QW).transpose(2, 1, 0, 3).reshape(NQ * P, KT * QW)
        )

    xT = [stripe_block(x[b].T) for b in range(B)]
    xT8 = [t.astype(fp8) for t in xT]
    xTb = [t.astype(bf16) for t in xT]
    in_maps = []
    for c in range(8):
        b, hg = c // HG, c % HG
        rows = slice(hg * NH * HD, (hg + 1) * NH * HD)
        m = {
            "xb": xTb[b],
            "wv": np.ascontiguousarray(Wv[rows].T).astype(bf16),
            "wp": np.ascontiguousarray(W_proj[:, rows].T).astype(bf16),
            "maskp": mask,
        }
        def head_major(w):
            # [D, NH*HD] -> [NH*D, HD]: contiguous per-head slabs
            return np.ascontiguousarray(
                w.reshape(D, NH, HD).transpose(1, 0, 2).reshape(NH * D, HD)
            )

        if USE_FP8:
            m["x8"] = xT8[b]
            m["wq"] = head_major(Wq[rows].T * WS).astype(fp8)
            m["wk"] = head_major(Wk[rows].T * WS).astype(fp8)
        else:
            m["wq"] = head_major(Wq[rows].T * WS).astype(bf16)
            m["wk"] = head_major(Wk[rows].T * WS).astype(bf16)
        in_maps.append(m)
    return in_maps


def run(x, W_qkv, W_proj, trace=False):
    nc = _get_nc()
    in_maps = make_in_maps(x, W_qkv, W_proj)
    res = run_bass_kernel_spmd(nc, in_maps, core_ids=list(range(8)), trace=trace)
    out = np.zeros((B, S, D), dtype=np.float32)
    for c in range(8):
        out[c // HG] += res.results[c]["y"].astype(np.float32)
    return out, res


def kernel(x, W_qkv, W_proj):
    out, _ = run(x, W_qkv, W_proj, trace=False)
    return out

